# revision 21
# baseline (speedup 1.0000x reference)
"""Multi-head self-attention (B=4, T=2048, D=1024, H=16) on 8 TRN2 NeuronCores.

Reference quirk: softmax normalizes over the QUERY axis (dim=2 of
[B,H,T1,T2]), i.e. attn[q,k] = exp(s[q,k]) / sum_q' exp(s[q',k]).

Sharding (fully SPMD, one NEFF for all 8 cores):
  core c -> batch b = c//2, head-group g = c%2 (8 heads = 512 cols of Wq/Wk/Wv).
  Host pre-slices AND pre-transposes per-core inputs (xT, wqT/wkT/wvT), runs
  the kernel, and stitches the 8 transposed [512, T] output shards back
  together (host-side transpose: device emits outT, avoiding PE transposes).

Device algorithm per core (v3 — software-pipelined, dense-PE schedule):
  1. x is DMAed as 32 [128,512] quarter-tiles (t-major order) so pair-0
     QT/KT projection can start ~6us in, overlapping the DMA tail.
     Prologue: QT/KT for pairs 0 AND 1 (PE work hidden under the x DMA),
     V[0:4]. Remaining V tiles dribble through pair 0's chunk stream;
     QT/KT of pair p+1 dribble through pair p's stream (p>=1).
  2. Per head-pair, per 128-wide key chunk:
       S = K @ Q^T [128 k, 1024 q] per (head, q-half) in PSUM; the two
       heads' score MMs are interleaved at adjacent tile_position row
       groups (0 / 64) so the PE can stream them concurrently,
       P = exp(SCALE * S) via ScalarE PSUM->SBUF (bf16),
       Z[k] row-sums via DVE tensor_reduce over P (keeps ScalarE lean),
       V'[k,:] = V[k,:] / Z[k] into persistent zero-padded vpad tiles,
       outT[d, q] += vpad^T @ P accumulated over 16 chunks in PSUM.
     Emission is pipelined: scores for chunk c+1 are issued between the
     exp and AV of chunk c so neither PE nor ACT queues behind the other.
  3. Epilogue per pair: acc -> SBUF copy -> DMA to outT rows (no transpose).
"""

import numpy as np

B, T, D, H = 4, 2048, 1024, 16
DH = D // H
SCALE = 1.0 / (DH**0.5)
N_CORES = 8
E = D // 2  # 512 output cols per core (8 heads)
N_PAIRS = 4  # head-pairs per core
N_DC = D // 128  # 8 contraction chunks for projections
N_KC = T // 128  # 16 key chunks
N_TQ = 4  # x quarter-tiles along t
QB = 1024  # exp free-dim block (2 PSUM banks)
V_PRE = 10  # V tiles projected in the prologue; rest dribbled

_built = None  # (nc,) cache so repeat kernel() calls skip rebuild/recompile


def _np_reference(x, padding_mask, Wq, Wk, Wv):
    """Pure-numpy fallback, used only if the mask is not all-ones."""
    x64 = x.astype(np.float64)
    Q = (x64 @ Wq.T.astype(np.float64)).reshape(B, T, H, DH).transpose(0, 2, 1, 3)
    K = (x64 @ Wk.T.astype(np.float64)).reshape(B, T, H, DH).transpose(0, 2, 1, 3)
    V = (x64 @ Wv.T.astype(np.float64)).reshape(B, T, H, DH).transpose(0, 2, 1, 3)
    s = np.einsum("bhqd,bhkd->bhqk", Q, K) * SCALE
    s = np.where(padding_mask[:, None, :, :] == 0, -np.inf, s)
    s = s - s.max(axis=2, keepdims=True)
    p = np.exp(s)
    p = p / p.sum(axis=2, keepdims=True)
    out = np.einsum("bhqk,bhkd->bhqd", p, V)
    return out.transpose(0, 2, 1, 3).reshape(B, T, D).astype(np.float32)


def _split_multi_waits(nc):
    """Walrus caps sync waits at 1 per instruction; Tile's tail drain can carry
    several. Move the extras onto single-wait drains appended to the previous
    basic block (same engine, earlier in program order)."""
    import concourse.mybir as mybir

    blocks = list(nc.m.functions[0].blocks)
    for bi, blk in enumerate(blocks):
        for inst in blk.instructions:
            if type(inst).__name__ not in ("InstDrain", "InstNoOp", "InstEventSemaphore"):
                continue
            si = inst.sync_info
            if si is not None and si.on_wait and len(si.on_wait) > 1:
                waits = list(si.on_wait)
                keep, extra = waits[-1], waits[:-1]
                assert all(w.wait_mode == "sem-ge-imm" for w in extra), extra
                si.on_wait = [keep]
                assert bi > 0, "multi-wait in first block"
                prev = blocks[bi - 1]
                for j, w in enumerate(extra):
                    d = mybir.InstDrain(
                        name=f"{inst.name}-ws{j}",
                        engine=inst.engine,
                        sync_info=mybir.SyncInfo(on_wait=[w], on_update=[]),
                    )
                    prev.add_instruction(d)


def _build_kernel(tc, xT, wqT, wkT, wvT, outT):
    import concourse.bass as bass  # noqa: F401
    import concourse.mybir as mybir

    nc = tc.nc
    FP = mybir.dt.float32
    FR = mybir.dt.float32r
    BF = mybir.dt.bfloat16
    Exp = mybir.ActivationFunctionType.Exp
    AX = mybir.AxisListType.X
    ADD = mybir.AluOpType.add

    # long-lived pools
    singles = tc.alloc_tile_pool(name="singles", bufs=1)
    xw = tc.alloc_tile_pool(name="xw", bufs=1)
    wp = tc.alloc_tile_pool(name="wp", bufs=3)
    qkv = tc.alloc_tile_pool(name="qkv", bufs=1)
    sps = tc.alloc_tile_pool(name="sps", bufs=2, space="PSUM")
    accps = tc.alloc_tile_pool(name="accps", bufs=1, space="PSUM")
    pp = tc.alloc_tile_pool(name="pp", bufs=8)
    zp = tc.alloc_tile_pool(name="zp", bufs=4)
    op = tc.alloc_tile_pool(name="op", bufs=2)

    # ---- loads: wq/wk first (pair-0 projections), x quarters t-major ----
    def load_w(wap, label):
        ws = []
        for dc in range(N_DC):
            t = wp.tile([128, E], BF, name=f"{label}{dc}", tag=f"w{dc}")
            nc.sync.dma_start(out=t, in_=wap[dc * 128 : (dc + 1) * 128, :])
            ws.append(t)
        return ws

    wq = load_w(wqT, "wq")

    xq = [[None] * N_TQ for _ in range(N_DC)]
    wk = wv = None
    for tq in range(N_TQ):
        for dc in range(N_DC):
            t = xw.tile([128, 512], BF, name=f"x{dc}_{tq}", tag=f"x{dc}_{tq}")
            nc.sync.dma_start(
                out=t, in_=xT[dc * 128 : (dc + 1) * 128, tq * 512 : (tq + 1) * 512]
            )
            xq[dc][tq] = t
        if tq == 0:
            wv = load_w(wvT, "wv")
            wk = load_w(wkT, "wk")

    # persistent zero-padded V' tiles: [parity][hi], data half written per chunk
    vpads = [[None, None], [None, None]]
    for par in range(2):
        for hi in range(2):
            vt = singles.tile([128, 128], BF, name=f"vp{par}{hi}")
            nc.gpsimd.memset(vt, 0.0)
            vpads[par][hi] = vt

    # warm-up: dummy matmuls on the zeroed tiles bridge the DMA latency so the
    # PE's HAM clock gate is released before the first real projection lands.
    wps = sps.tile([128, QB], FP, name="warm", tag="s")
    for i in range(64):
        nc.tensor.matmul(
            wps[:, 0:128], vpads[0][0], vpads[1][0], start=(i == 0), stop=(i == 63)
        )

    # ---- projection emitters (psum borrowed from the S pool tag).
    # Up to two [128,512] blocks share one psum borrow so dribbled projections
    # insert into the S rotation as rarely as possible.
    def eT_block_mms(ws, pair, tt, pshalf):
        for dc in range(N_DC):
            nc.tensor.matmul(
                pshalf,
                ws[dc][:, pair * 128 : (pair + 1) * 128],
                xq[dc][tt],
                start=(dc == 0),
                stop=(dc == N_DC - 1),
            )

    def v_block_mms(tt, pshalf):
        tq, to = divmod(tt, 4)
        for dc in range(N_DC):
            nc.tensor.matmul(
                pshalf,
                xq[dc][tq][:, to * 128 : (to + 1) * 128],
                wv[dc],
                start=(dc == 0),
                stop=(dc == N_DC - 1),
            )

    def project_eT(ws, pair, tts):
        """1-2 t-blocks of a QT/KT pair tile through one psum borrow."""
        ets = QT if ws is wq else KT
        ps = sps.tile([128, QB], FP, name=f"ps_e{pair}_{tts[0]}", tag="s")
        for i, tt in enumerate(tts):
            eT_block_mms(ws, pair, tt, ps[:, i * 512 : (i + 1) * 512])
        et = ets[pair]
        if len(tts) == 2 and tts[1] == tts[0] + 1:
            nc.vector.tensor_copy(et[:, tts[0] * 512 : (tts[0] + 2) * 512], ps)
        else:
            for i, tt in enumerate(tts):
                nc.vector.tensor_copy(
                    et[:, tt * 512 : (tt + 1) * 512], ps[:, i * 512 : (i + 1) * 512]
                )

    def project_v(tts):
        """1-2 V tiles through one psum borrow."""
        ps = sps.tile([128, QB], FP, name=f"ps_v{tts[0]}", tag="s")
        for i, tt in enumerate(tts):
            v_block_mms(tt, ps[:, i * 512 : (i + 1) * 512])
        for i, tt in enumerate(tts):
            v = qkv.tile([128, E], BF, name=f"v{tt}", tag=f"v{tt}")
            nc.vector.tensor_copy(v, ps[:, i * 512 : (i + 1) * 512])
            V[tt] = v

    QT = [None] * N_PAIRS
    KT = [None] * N_PAIRS
    V = [None] * N_KC

    def alloc_pair(p):
        QT[p] = qkv.tile([128, T], BF, name=f"qt{p}", tag=f"qt{p}")
        KT[p] = qkv.tile([128, T], BF, name=f"kt{p}", tag=f"kt{p}")

    # prologue: only what scores(0,0)/exp(0,0) truly gate on — all of QT0
    # (every chunk reads all queries), KT0's chunk-0 block, early V tiles.
    # KT0's later blocks (needed from chunk 4 on) dribble into pair 0.
    alloc_pair(0)
    project_eT(wq, 0, (0,))
    project_v((0, 1))
    project_v((2, 3))
    project_eT(wk, 0, (0,))
    project_v((4, 5))
    project_v((6, 7))
    project_eT(wq, 0, (1,))
    project_v((8, 9))
    project_eT(wq, 0, (2, 3))

    # dribble schedule: work[(p, c)] -> list of zero-arg emitters.
    # Single-block borrows, at most one per chunk, kept off the fragile
    # pair-start chunks (c<2). Own KT t1-3 land just before their first use
    # (chunk 4*tt); the next pair's QT + KT-t0 fill the tail chunks.
    work = {}

    def add_work(p, c, fn):
        work.setdefault((p, c), []).append(fn)

    def proj_item(ws, q, tts):
        return lambda: project_eT(ws, q, tts)

    for p in range(N_PAIRS):
        add_work(p, 2, proj_item(wk, p, (1,)))
        add_work(p, 5, proj_item(wk, p, (2,)))
        add_work(p, 8, proj_item(wk, p, (3,)))
        if p < N_PAIRS - 1:
            add_work(p, 9, (lambda q: (lambda: alloc_pair(q)))(p + 1))
            add_work(p, 10, proj_item(wq, p + 1, (0,)))
            add_work(p, 11, proj_item(wq, p + 1, (1,)))
            add_work(p, 12, proj_item(wq, p + 1, (2,)))
            add_work(p, 13, proj_item(wq, p + 1, (3,)))
            add_work(p, 14, proj_item(wk, p + 1, (0,)))
    for i, c in enumerate((3, 4, 6)):
        tt = V_PRE + 2 * i  # V[tt], V[tt+1] during pair 0, just-in-time
        add_work(0, c, (lambda t2: (lambda: project_v((t2, t2 + 1))))(tt))

    # ---- attention emitters ----
    def scores_half(p, c, hi):
        """S tiles for one head of the pair: 2x [128k, 1024q] psum."""
        base = hi * 64
        out = {}
        for qb in range(2):
            s = sps.tile([128, QB], FP, name=f"s_{p}_{c}_{hi}_{qb}", tag="s")
            for qt in range(2):
                q0 = qb * QB + qt * 512
                nc.tensor.matmul(
                    s[:, qt * 512 : (qt + 1) * 512],
                    KT[p][base : base + 64, c * 128 : (c + 1) * 128],
                    QT[p][base : base + 64, q0 : q0 + 512],
                    start=True,
                    stop=True,
                    tile_position=(base, 0),
                )
            out[qb] = s
        return out

    def exps_half(p, c, hi, stiles, ptiles, zs):
        """Two exps for one head. qb0's row-sum goes to DVE (tensor_reduce,
        hidden under qb1's exp); qb1 uses the ScalarE accumulator."""
        for qb in range(2):
            pt = pp.tile([128, QB], BF, name=f"p_{p}_{c}_{hi}_{qb}", tag="p")
            nc.scalar.activation(
                out=pt,
                in_=stiles[(hi, qb)],
                func=Exp,
                scale=SCALE,
                accum_out=zs[:, 2 * hi + 1 : 2 * hi + 2] if qb == 1 else None,
            )
            if qb == 0:
                nc.vector.tensor_reduce(
                    zs[:, 2 * hi : 2 * hi + 1], pt, axis=AX, op=ADD
                )
            ptiles[(hi, qb)] = pt

    def zchain_half(p, c, hi, zs):
        """Z -> 1/Z -> scaled V' for one head; needs only that head's exps."""
        za = zp.tile([128, 1], FP, name=f"za_{p}_{c}_{hi}", tag=f"za{hi}")
        nc.vector.tensor_add(za, zs[:, 2 * hi : 2 * hi + 1], zs[:, 2 * hi + 1 : 2 * hi + 2])
        rz = zp.tile([128, 1], FP, name=f"rz_{p}_{c}_{hi}", tag=f"rz{hi}")
        nc.vector.reciprocal(out=rz, in_=za)
        vt = vpads[c % 2][hi]
        lo = hi * 64
        nc.vector.tensor_scalar_mul(
            vt[:, lo : lo + 64],
            V[c][:, p * 128 + lo : p * 128 + lo + 64],
            rz,
        )
        return vt

    def av_half(p, c, acc, vt, ptiles, hi):
        for qb in range(2):
            for qt in range(2):
                nc.tensor.matmul(
                    acc[qb][:, qt * 512 : (qt + 1) * 512],
                    vt,
                    ptiles[(hi, qb)][:, qt * 512 : (qt + 1) * 512],
                    start=(c == 0 and hi == 0),
                    stop=(c == N_KC - 1 and hi == 1),
                )

    # ---- pipelined main loop ----
    stiles = {}
    for hi in range(2):
        for qb, s in scores_half(0, 0, hi).items():
            stiles[(hi, qb)] = s

    for p in range(N_PAIRS):
        acc = [
            accps.tile([128, QB], FP, name=f"acc{qb}_{p}", tag=f"acc{qb}")
            for qb in range(2)
        ]
        for c in range(N_KC):
            zs = zp.tile([128, 4], FP, name=f"zs_{p}_{c}", tag="zs")
            ptiles = {}
            nxt = (p, c + 1) if c + 1 < N_KC else (p + 1, 0)
            # head 0: exps -> Z chain -> next-chunk scores -> AV
            exps_half(p, c, 0, stiles, ptiles, zs)
            vt0 = zchain_half(p, c, 0, zs)
            nstiles = {}
            if nxt[0] < N_PAIRS:
                for qb, s in scores_half(*nxt, 0).items():
                    nstiles[(0, qb)] = s
            av_half(p, c, acc, vt0, ptiles, 0)
            # head 1 likewise, overlapping head 0's AV with its exps
            exps_half(p, c, 1, stiles, ptiles, zs)
            vt1 = zchain_half(p, c, 1, zs)
            if nxt[0] < N_PAIRS:
                for qb, s in scores_half(*nxt, 1).items():
                    nstiles[(1, qb)] = s
            av_half(p, c, acc, vt1, ptiles, 1)
            stiles = nstiles
            for fn in work.get((p, c), []):
                fn()
        # epilogue: outT rows for this pair (host transposes back)
        for qb in range(2):
            ot = op.tile([128, QB], FP, name=f"ot_{p}_{qb}", tag="ot")
            nc.vector.tensor_copy(ot, acc[qb])
            nc.sync.dma_start(
                out=outT[p * 128 : (p + 1) * 128, qb * QB : (qb + 1) * QB], in_=ot
            )

    for pool in (op, zp, pp, accps, sps, qkv, wp, xw, singles):
        pool.release()


def build():
    import concourse.bacc as bacc
    import concourse.mybir as mybir
    import concourse.tile as tile

    nc = bacc.Bacc("TRN2", target_bir_lowering=False, debug=False)
    FP = mybir.dt.float32
    BF = mybir.dt.bfloat16
    xT = nc.dram_tensor("xT", [D, T], BF, kind="ExternalInput").ap()
    wqT = nc.dram_tensor("wqT", [D, E], BF, kind="ExternalInput").ap()
    wkT = nc.dram_tensor("wkT", [D, E], BF, kind="ExternalInput").ap()
    wvT = nc.dram_tensor("wvT", [D, E], BF, kind="ExternalInput").ap()
    outT = nc.dram_tensor("outT", [E, T], FP, kind="ExternalOutput").ap()
    with tile.TileContext(nc) as tc:
        _build_kernel(tc, xT, wqT, wkT, wvT, outT)
    nc.compile()
    _split_multi_waits(nc)
    return nc


def _get_nc():
    global _built
    if _built is None:
        _built = build()
    return _built


def make_in_maps(x, Wq, Wk, Wv):
    import ml_dtypes

    bf16 = ml_dtypes.bfloat16
    in_maps = []
    for c in range(N_CORES):
        b, g = divmod(c, 2)
        e0 = E * g
        in_maps.append(
            {
                "xT": np.ascontiguousarray(x[b].T).astype(bf16),
                "wqT": np.ascontiguousarray(Wq[e0 : e0 + E, :].T).astype(bf16),
                "wkT": np.ascontiguousarray(Wk[e0 : e0 + E, :].T).astype(bf16),
                "wvT": np.ascontiguousarray(Wv[e0 : e0 + E, :].T).astype(bf16),
            }
        )
    return in_maps


def assemble_out(results):
    out = np.empty((B, T, D), np.float32)
    for c in range(N_CORES):
        b, g = divmod(c, 2)
        e0 = E * g
        out[b][:, e0 : e0 + E] = results[c]["outT"].T
    return out


def kernel(x, padding_mask, Wq, Wk, Wv):
    x = np.asarray(x, dtype=np.float32)
    padding_mask = np.asarray(padding_mask, dtype=np.float32)
    Wq = np.asarray(Wq, dtype=np.float32)
    Wk = np.asarray(Wk, dtype=np.float32)
    Wv = np.asarray(Wv, dtype=np.float32)
    if not np.all(padding_mask == 1.0):
        return _np_reference(x, padding_mask, Wq, Wk, Wv)

    from concourse.bass_utils import run_bass_kernel_spmd

    nc = _get_nc()
    in_maps = make_in_maps(x, Wq, Wk, Wv)
    res = run_bass_kernel_spmd(nc, in_maps, list(range(N_CORES)))
    return assemble_out(res.results)


# revision 22
# speedup vs baseline: 1.0014x; 1.0014x over previous
"""Multi-head self-attention (B=4, T=2048, D=1024, H=16) on 8 TRN2 NeuronCores.

Reference quirk: softmax normalizes over the QUERY axis (dim=2 of
[B,H,T1,T2]), i.e. attn[q,k] = exp(s[q,k]) / sum_q' exp(s[q',k]).

Sharding (fully SPMD, one NEFF for all 8 cores):
  core c -> batch b = c//2, head-group g = c%2 (8 heads = 512 cols of Wq/Wk/Wv).
  Host pre-slices AND pre-transposes per-core inputs (xT, wqT/wkT/wvT), runs
  the kernel, and stitches the 8 transposed [512, T] output shards back
  together (host-side transpose: device emits outT, avoiding PE transposes).

Device algorithm per core (v3 — software-pipelined, dense-PE schedule):
  1. x is DMAed as 32 [128,512] quarter-tiles (t-major order) so pair-0
     QT/KT projection can start ~6us in, overlapping the DMA tail.
     Prologue: QT/KT for pairs 0 AND 1 (PE work hidden under the x DMA),
     V[0:4]. Remaining V tiles dribble through pair 0's chunk stream;
     QT/KT of pair p+1 dribble through pair p's stream (p>=1).
  2. Per head-pair, per 128-wide key chunk:
       S = K @ Q^T [128 k, 1024 q] per (head, q-half) in PSUM; the two
       heads' score MMs are interleaved at adjacent tile_position row
       groups (0 / 64) so the PE can stream them concurrently,
       P = exp(SCALE * S) via ScalarE PSUM->SBUF (bf16),
       Z[k] row-sums via DVE tensor_reduce over P (keeps ScalarE lean),
       V'[k,:] = V[k,:] / Z[k] into persistent zero-padded vpad tiles,
       outT[d, q] += vpad^T @ P accumulated over 16 chunks in PSUM.
     Emission is pipelined: scores for chunk c+1 are issued between the
     exp and AV of chunk c so neither PE nor ACT queues behind the other.
  3. Epilogue per pair: acc -> SBUF copy -> DMA to outT rows (no transpose).
"""

import numpy as np

B, T, D, H = 4, 2048, 1024, 16
DH = D // H
SCALE = 1.0 / (DH**0.5)
N_CORES = 8
E = D // 2  # 512 output cols per core (8 heads)
N_PAIRS = 4  # head-pairs per core
N_DC = D // 128  # 8 contraction chunks for projections
N_KC = T // 128  # 16 key chunks
N_TQ = 4  # x quarter-tiles along t
QB = 1024  # exp free-dim block (2 PSUM banks)
V_PRE = 10  # V tiles projected in the prologue; rest dribbled

_built = None  # (nc,) cache so repeat kernel() calls skip rebuild/recompile


def _np_reference(x, padding_mask, Wq, Wk, Wv):
    """Pure-numpy fallback, used only if the mask is not all-ones."""
    x64 = x.astype(np.float64)
    Q = (x64 @ Wq.T.astype(np.float64)).reshape(B, T, H, DH).transpose(0, 2, 1, 3)
    K = (x64 @ Wk.T.astype(np.float64)).reshape(B, T, H, DH).transpose(0, 2, 1, 3)
    V = (x64 @ Wv.T.astype(np.float64)).reshape(B, T, H, DH).transpose(0, 2, 1, 3)
    s = np.einsum("bhqd,bhkd->bhqk", Q, K) * SCALE
    s = np.where(padding_mask[:, None, :, :] == 0, -np.inf, s)
    s = s - s.max(axis=2, keepdims=True)
    p = np.exp(s)
    p = p / p.sum(axis=2, keepdims=True)
    out = np.einsum("bhqk,bhkd->bhqd", p, V)
    return out.transpose(0, 2, 1, 3).reshape(B, T, D).astype(np.float32)


def _split_multi_waits(nc):
    """Walrus caps sync waits at 1 per instruction; Tile's tail drain can carry
    several. Move the extras onto single-wait drains appended to the previous
    basic block (same engine, earlier in program order)."""
    import concourse.mybir as mybir

    blocks = list(nc.m.functions[0].blocks)
    for bi, blk in enumerate(blocks):
        for inst in blk.instructions:
            if type(inst).__name__ not in ("InstDrain", "InstNoOp", "InstEventSemaphore"):
                continue
            si = inst.sync_info
            if si is not None and si.on_wait and len(si.on_wait) > 1:
                waits = list(si.on_wait)
                keep, extra = waits[-1], waits[:-1]
                assert all(w.wait_mode == "sem-ge-imm" for w in extra), extra
                si.on_wait = [keep]
                assert bi > 0, "multi-wait in first block"
                prev = blocks[bi - 1]
                for j, w in enumerate(extra):
                    d = mybir.InstDrain(
                        name=f"{inst.name}-ws{j}",
                        engine=inst.engine,
                        sync_info=mybir.SyncInfo(on_wait=[w], on_update=[]),
                    )
                    prev.add_instruction(d)


def _build_kernel(tc, xT, wqT, wkT, wvT, outT):
    import concourse.bass as bass  # noqa: F401
    import concourse.mybir as mybir

    nc = tc.nc
    FP = mybir.dt.float32
    FR = mybir.dt.float32r
    BF = mybir.dt.bfloat16
    Exp = mybir.ActivationFunctionType.Exp
    AX = mybir.AxisListType.X
    ADD = mybir.AluOpType.add

    # long-lived pools
    singles = tc.alloc_tile_pool(name="singles", bufs=1)
    xw = tc.alloc_tile_pool(name="xw", bufs=1)
    wp = tc.alloc_tile_pool(name="wp", bufs=3)
    qkv = tc.alloc_tile_pool(name="qkv", bufs=1)
    sps = tc.alloc_tile_pool(name="sps", bufs=2, space="PSUM")
    accps = tc.alloc_tile_pool(name="accps", bufs=1, space="PSUM")
    pp = tc.alloc_tile_pool(name="pp", bufs=8)
    zp = tc.alloc_tile_pool(name="zp", bufs=4)
    op = tc.alloc_tile_pool(name="op", bufs=2)

    # ---- loads: wq/wk first (pair-0 projections), x quarters t-major ----
    def load_w(wap, label):
        ws = []
        for dc in range(N_DC):
            t = wp.tile([128, E], BF, name=f"{label}{dc}", tag=f"w{dc}")
            nc.sync.dma_start(out=t, in_=wap[dc * 128 : (dc + 1) * 128, :])
            ws.append(t)
        return ws

    wq = load_w(wqT, "wq")

    xq = [[None] * N_TQ for _ in range(N_DC)]
    wk = wv = None
    for tq in range(N_TQ):
        for dc in range(N_DC):
            t = xw.tile([128, 512], BF, name=f"x{dc}_{tq}", tag=f"x{dc}_{tq}")
            nc.sync.dma_start(
                out=t, in_=xT[dc * 128 : (dc + 1) * 128, tq * 512 : (tq + 1) * 512]
            )
            xq[dc][tq] = t
        if tq == 0:
            wv = load_w(wvT, "wv")
            wk = load_w(wkT, "wk")

    # persistent zero-padded V' tiles: [parity][hi], data half written per chunk
    vpads = [[None, None], [None, None]]
    for par in range(2):
        for hi in range(2):
            vt = singles.tile([128, 128], BF, name=f"vp{par}{hi}")
            nc.gpsimd.memset(vt, 0.0)
            vpads[par][hi] = vt

    # warm-up: dummy matmuls on the zeroed tiles bridge the DMA latency so the
    # PE's HAM clock gate is released before the first real projection lands.
    wps = sps.tile([128, QB], FP, name="warm", tag="s")
    for i in range(64):
        nc.tensor.matmul(
            wps[:, 0:128], vpads[0][0], vpads[1][0], start=(i == 0), stop=(i == 63)
        )

    # ---- projection emitters (psum borrowed from the S pool tag).
    # Up to two [128,512] blocks share one psum borrow so dribbled projections
    # insert into the S rotation as rarely as possible.
    def eT_block_mms(ws, pair, tt, pshalf):
        for dc in range(N_DC):
            nc.tensor.matmul(
                pshalf,
                ws[dc][:, pair * 128 : (pair + 1) * 128],
                xq[dc][tt],
                start=(dc == 0),
                stop=(dc == N_DC - 1),
            )

    def v_block_mms(tt, pshalf):
        tq, to = divmod(tt, 4)
        for dc in range(N_DC):
            nc.tensor.matmul(
                pshalf,
                xq[dc][tq][:, to * 128 : (to + 1) * 128],
                wv[dc],
                start=(dc == 0),
                stop=(dc == N_DC - 1),
            )

    def project_eT(ws, pair, tts):
        """1-2 t-blocks of a QT/KT pair tile through one psum borrow."""
        ets = QT if ws is wq else KT
        ps = sps.tile([128, QB], FP, name=f"ps_e{pair}_{tts[0]}", tag="s")
        for i, tt in enumerate(tts):
            eT_block_mms(ws, pair, tt, ps[:, i * 512 : (i + 1) * 512])
        et = ets[pair]
        if len(tts) == 2 and tts[1] == tts[0] + 1:
            nc.vector.tensor_copy(et[:, tts[0] * 512 : (tts[0] + 2) * 512], ps)
        else:
            for i, tt in enumerate(tts):
                nc.vector.tensor_copy(
                    et[:, tt * 512 : (tt + 1) * 512], ps[:, i * 512 : (i + 1) * 512]
                )

    def project_v(tts):
        """1-2 V tiles through one psum borrow."""
        ps = sps.tile([128, QB], FP, name=f"ps_v{tts[0]}", tag="s")
        for i, tt in enumerate(tts):
            v_block_mms(tt, ps[:, i * 512 : (i + 1) * 512])
        for i, tt in enumerate(tts):
            v = qkv.tile([128, E], BF, name=f"v{tt}", tag=f"v{tt}")
            nc.vector.tensor_copy(v, ps[:, i * 512 : (i + 1) * 512])
            V[tt] = v

    QT = [None] * N_PAIRS
    KT = [None] * N_PAIRS
    V = [None] * N_KC

    def alloc_pair(p):
        QT[p] = qkv.tile([128, T], BF, name=f"qt{p}", tag=f"qt{p}")
        KT[p] = qkv.tile([128, T], BF, name=f"kt{p}", tag=f"kt{p}")

    # Two-phase design: ALL projections run in the prologue, ordered by the
    # x quarter they consume so the (warm, dense) PE tracks the DMA stream.
    # In-stream projection borrows proved to cost 2-4us of pipeline
    # disruption each, far more than their PE time — so the chunk stream
    # below runs with zero psum-rotation insertions.
    for p in range(N_PAIRS):
        alloc_pair(p)
    for tq in range(N_TQ):
        for p in range(N_PAIRS):
            project_eT(wq, p, (tq,))
            project_eT(wk, p, (tq,))
        project_v((4 * tq, 4 * tq + 1))
        project_v((4 * tq + 2, 4 * tq + 3))

    work = {}

    # ---- attention emitters ----
    def scores_half(p, c, hi):
        """S tiles for one head of the pair: 2x [128k, 1024q] psum."""
        base = hi * 64
        out = {}
        for qb in range(2):
            s = sps.tile([128, QB], FP, name=f"s_{p}_{c}_{hi}_{qb}", tag="s")
            for qt in range(2):
                q0 = qb * QB + qt * 512
                nc.tensor.matmul(
                    s[:, qt * 512 : (qt + 1) * 512],
                    KT[p][base : base + 64, c * 128 : (c + 1) * 128],
                    QT[p][base : base + 64, q0 : q0 + 512],
                    start=True,
                    stop=True,
                    tile_position=(base, 0),
                )
            out[qb] = s
        return out

    def exps_half(p, c, hi, stiles, ptiles, zs):
        """Two exps for one head. qb0's row-sum goes to DVE (tensor_reduce,
        hidden under qb1's exp); qb1 uses the ScalarE accumulator."""
        for qb in range(2):
            pt = pp.tile([128, QB], BF, name=f"p_{p}_{c}_{hi}_{qb}", tag="p")
            nc.scalar.activation(
                out=pt,
                in_=stiles[(hi, qb)],
                func=Exp,
                scale=SCALE,
                accum_out=zs[:, 2 * hi + 1 : 2 * hi + 2] if qb == 1 else None,
            )
            if qb == 0:
                nc.vector.tensor_reduce(
                    zs[:, 2 * hi : 2 * hi + 1], pt, axis=AX, op=ADD
                )
            ptiles[(hi, qb)] = pt

    def zchain_half(p, c, hi, zs):
        """Z -> 1/Z -> scaled V' for one head; needs only that head's exps."""
        za = zp.tile([128, 1], FP, name=f"za_{p}_{c}_{hi}", tag=f"za{hi}")
        nc.vector.tensor_add(za, zs[:, 2 * hi : 2 * hi + 1], zs[:, 2 * hi + 1 : 2 * hi + 2])
        rz = zp.tile([128, 1], FP, name=f"rz_{p}_{c}_{hi}", tag=f"rz{hi}")
        nc.vector.reciprocal(out=rz, in_=za)
        vt = vpads[c % 2][hi]
        lo = hi * 64
        nc.vector.tensor_scalar_mul(
            vt[:, lo : lo + 64],
            V[c][:, p * 128 + lo : p * 128 + lo + 64],
            rz,
        )
        return vt

    def av_half(p, c, acc, vt, ptiles, hi):
        for qb in range(2):
            for qt in range(2):
                nc.tensor.matmul(
                    acc[qb][:, qt * 512 : (qt + 1) * 512],
                    vt,
                    ptiles[(hi, qb)][:, qt * 512 : (qt + 1) * 512],
                    start=(c == 0 and hi == 0),
                    stop=(c == N_KC - 1 and hi == 1),
                )

    # ---- pipelined main loop ----
    stiles = {}
    for hi in range(2):
        for qb, s in scores_half(0, 0, hi).items():
            stiles[(hi, qb)] = s

    for p in range(N_PAIRS):
        acc = [
            accps.tile([128, QB], FP, name=f"acc{qb}_{p}", tag=f"acc{qb}")
            for qb in range(2)
        ]
        for c in range(N_KC):
            zs = zp.tile([128, 4], FP, name=f"zs_{p}_{c}", tag="zs")
            ptiles = {}
            nxt = (p, c + 1) if c + 1 < N_KC else (p + 1, 0)
            # head 0: exps -> Z chain -> next-chunk scores -> AV
            exps_half(p, c, 0, stiles, ptiles, zs)
            vt0 = zchain_half(p, c, 0, zs)
            nstiles = {}
            if nxt[0] < N_PAIRS:
                for qb, s in scores_half(*nxt, 0).items():
                    nstiles[(0, qb)] = s
            av_half(p, c, acc, vt0, ptiles, 0)
            # head 1 likewise, overlapping head 0's AV with its exps
            exps_half(p, c, 1, stiles, ptiles, zs)
            vt1 = zchain_half(p, c, 1, zs)
            if nxt[0] < N_PAIRS:
                for qb, s in scores_half(*nxt, 1).items():
                    nstiles[(1, qb)] = s
            av_half(p, c, acc, vt1, ptiles, 1)
            stiles = nstiles
            for fn in work.get((p, c), []):
                fn()
        # epilogue: outT rows for this pair (host transposes back)
        for qb in range(2):
            ot = op.tile([128, QB], FP, name=f"ot_{p}_{qb}", tag="ot")
            nc.vector.tensor_copy(ot, acc[qb])
            nc.sync.dma_start(
                out=outT[p * 128 : (p + 1) * 128, qb * QB : (qb + 1) * QB], in_=ot
            )

    for pool in (op, zp, pp, accps, sps, qkv, wp, xw, singles):
        pool.release()


def build():
    import concourse.bacc as bacc
    import concourse.mybir as mybir
    import concourse.tile as tile

    nc = bacc.Bacc("TRN2", target_bir_lowering=False, debug=False)
    FP = mybir.dt.float32
    BF = mybir.dt.bfloat16
    xT = nc.dram_tensor("xT", [D, T], BF, kind="ExternalInput").ap()
    wqT = nc.dram_tensor("wqT", [D, E], BF, kind="ExternalInput").ap()
    wkT = nc.dram_tensor("wkT", [D, E], BF, kind="ExternalInput").ap()
    wvT = nc.dram_tensor("wvT", [D, E], BF, kind="ExternalInput").ap()
    outT = nc.dram_tensor("outT", [E, T], FP, kind="ExternalOutput").ap()
    with tile.TileContext(nc) as tc:
        _build_kernel(tc, xT, wqT, wkT, wvT, outT)
    nc.compile()
    _split_multi_waits(nc)
    return nc


def _get_nc():
    global _built
    if _built is None:
        _built = build()
    return _built


def make_in_maps(x, Wq, Wk, Wv):
    import ml_dtypes

    bf16 = ml_dtypes.bfloat16
    in_maps = []
    for c in range(N_CORES):
        b, g = divmod(c, 2)
        e0 = E * g
        in_maps.append(
            {
                "xT": np.ascontiguousarray(x[b].T).astype(bf16),
                "wqT": np.ascontiguousarray(Wq[e0 : e0 + E, :].T).astype(bf16),
                "wkT": np.ascontiguousarray(Wk[e0 : e0 + E, :].T).astype(bf16),
                "wvT": np.ascontiguousarray(Wv[e0 : e0 + E, :].T).astype(bf16),
            }
        )
    return in_maps


def assemble_out(results):
    out = np.empty((B, T, D), np.float32)
    for c in range(N_CORES):
        b, g = divmod(c, 2)
        e0 = E * g
        out[b][:, e0 : e0 + E] = results[c]["outT"].T
    return out


def kernel(x, padding_mask, Wq, Wk, Wv):
    x = np.asarray(x, dtype=np.float32)
    padding_mask = np.asarray(padding_mask, dtype=np.float32)
    Wq = np.asarray(Wq, dtype=np.float32)
    Wk = np.asarray(Wk, dtype=np.float32)
    Wv = np.asarray(Wv, dtype=np.float32)
    if not np.all(padding_mask == 1.0):
        return _np_reference(x, padding_mask, Wq, Wk, Wv)

    from concourse.bass_utils import run_bass_kernel_spmd

    nc = _get_nc()
    in_maps = make_in_maps(x, Wq, Wk, Wv)
    res = run_bass_kernel_spmd(nc, in_maps, list(range(N_CORES)))
    return assemble_out(res.results)


# revision 24
# speedup vs baseline: 1.0462x; 1.0448x over previous
"""Multi-head self-attention (B=4, T=2048, D=1024, H=16) on 8 TRN2 NeuronCores.

Reference quirk: softmax normalizes over the QUERY axis (dim=2 of
[B,H,T1,T2]), i.e. attn[q,k] = exp(s[q,k]) / sum_q' exp(s[q',k]).

Sharding (fully SPMD, one NEFF for all 8 cores):
  core c -> batch b = c//2, head-group g = c%2 (8 heads = 512 cols of Wq/Wk/Wv).
  Host pre-slices AND pre-transposes per-core inputs (xT, wqT/wkT/wvT), runs
  the kernel, and stitches the 8 transposed [512, T] output shards back
  together (host-side transpose: device emits outT, avoiding PE transposes).

Device algorithm per core (v3 — software-pipelined, dense-PE schedule):
  1. x is DMAed as 32 [128,512] quarter-tiles (t-major order) so pair-0
     QT/KT projection can start ~6us in, overlapping the DMA tail.
     Prologue: QT/KT for pairs 0 AND 1 (PE work hidden under the x DMA),
     V[0:4]. Remaining V tiles dribble through pair 0's chunk stream;
     QT/KT of pair p+1 dribble through pair p's stream (p>=1).
  2. Per head-pair, per 128-wide key chunk:
       S = K @ Q^T [128 k, 1024 q] per (head, q-half) in PSUM; the two
       heads' score MMs are interleaved at adjacent tile_position row
       groups (0 / 64) so the PE can stream them concurrently,
       P = exp(SCALE * S) via ScalarE PSUM->SBUF (bf16),
       Z[k] row-sums via DVE tensor_reduce over P (keeps ScalarE lean),
       V'[k,:] = V[k,:] / Z[k] into persistent zero-padded vpad tiles,
       outT[d, q] += vpad^T @ P accumulated over 16 chunks in PSUM.
     Emission is pipelined: scores for chunk c+1 are issued between the
     exp and AV of chunk c so neither PE nor ACT queues behind the other.
  3. Epilogue per pair: acc -> SBUF copy -> DMA to outT rows (no transpose).
"""

import numpy as np

B, T, D, H = 4, 2048, 1024, 16
DH = D // H
SCALE = 1.0 / (DH**0.5)
N_CORES = 8
E = D // 2  # 512 output cols per core (8 heads)
N_PAIRS = 4  # head-pairs per core
N_DC = D // 128  # 8 contraction chunks for projections
N_KC = T // 128  # 16 key chunks
N_TQ = 4  # x quarter-tiles along t
QB = 1024  # exp free-dim block (2 PSUM banks)
V_PRE = 10  # V tiles projected in the prologue; rest dribbled

_built = None  # (nc,) cache so repeat kernel() calls skip rebuild/recompile


def _np_reference(x, padding_mask, Wq, Wk, Wv):
    """Pure-numpy fallback, used only if the mask is not all-ones."""
    x64 = x.astype(np.float64)
    Q = (x64 @ Wq.T.astype(np.float64)).reshape(B, T, H, DH).transpose(0, 2, 1, 3)
    K = (x64 @ Wk.T.astype(np.float64)).reshape(B, T, H, DH).transpose(0, 2, 1, 3)
    V = (x64 @ Wv.T.astype(np.float64)).reshape(B, T, H, DH).transpose(0, 2, 1, 3)
    s = np.einsum("bhqd,bhkd->bhqk", Q, K) * SCALE
    s = np.where(padding_mask[:, None, :, :] == 0, -np.inf, s)
    s = s - s.max(axis=2, keepdims=True)
    p = np.exp(s)
    p = p / p.sum(axis=2, keepdims=True)
    out = np.einsum("bhqk,bhkd->bhqd", p, V)
    return out.transpose(0, 2, 1, 3).reshape(B, T, D).astype(np.float32)


def _split_multi_waits(nc):
    """Walrus caps sync waits at 1 per instruction; Tile's tail drain can carry
    several. Move the extras onto single-wait drains appended to the previous
    basic block (same engine, earlier in program order)."""
    import concourse.mybir as mybir

    blocks = list(nc.m.functions[0].blocks)
    for bi, blk in enumerate(blocks):
        for inst in blk.instructions:
            if type(inst).__name__ not in ("InstDrain", "InstNoOp", "InstEventSemaphore"):
                continue
            si = inst.sync_info
            if si is not None and si.on_wait and len(si.on_wait) > 1:
                waits = list(si.on_wait)
                keep, extra = waits[-1], waits[:-1]
                assert all(w.wait_mode == "sem-ge-imm" for w in extra), extra
                si.on_wait = [keep]
                assert bi > 0, "multi-wait in first block"
                prev = blocks[bi - 1]
                for j, w in enumerate(extra):
                    d = mybir.InstDrain(
                        name=f"{inst.name}-ws{j}",
                        engine=inst.engine,
                        sync_info=mybir.SyncInfo(on_wait=[w], on_update=[]),
                    )
                    prev.add_instruction(d)


def _build_kernel(tc, xT, wqT, wkT, wvT, outT):
    import concourse.bass as bass  # noqa: F401
    import concourse.mybir as mybir

    nc = tc.nc
    FP = mybir.dt.float32
    FR = mybir.dt.float32r
    BF = mybir.dt.bfloat16
    Exp = mybir.ActivationFunctionType.Exp
    AX = mybir.AxisListType.X
    ADD = mybir.AluOpType.add

    # long-lived pools
    singles = tc.alloc_tile_pool(name="singles", bufs=1)
    xw = tc.alloc_tile_pool(name="xw", bufs=1)
    wp = tc.alloc_tile_pool(name="wp", bufs=3)
    qkv = tc.alloc_tile_pool(name="qkv", bufs=1)
    sps = tc.alloc_tile_pool(name="sps", bufs=2, space="PSUM")
    accps = tc.alloc_tile_pool(name="accps", bufs=1, space="PSUM")
    pp = tc.alloc_tile_pool(name="pp", bufs=8)
    zp = tc.alloc_tile_pool(name="zp", bufs=4)
    op = tc.alloc_tile_pool(name="op", bufs=2)

    # ---- loads: wq/wk first (pair-0 projections), x quarters t-major ----
    def load_w(wap, label):
        ws = []
        for dc in range(N_DC):
            t = wp.tile([128, E], BF, name=f"{label}{dc}", tag=f"w{dc}")
            nc.sync.dma_start(out=t, in_=wap[dc * 128 : (dc + 1) * 128, :])
            ws.append(t)
        return ws

    wq = load_w(wqT, "wq")

    xq = [[None] * N_TQ for _ in range(N_DC)]
    wk = wv = None
    for tq in range(N_TQ):
        for dc in range(N_DC):
            t = xw.tile([128, 512], BF, name=f"x{dc}_{tq}", tag=f"x{dc}_{tq}")
            nc.sync.dma_start(
                out=t, in_=xT[dc * 128 : (dc + 1) * 128, tq * 512 : (tq + 1) * 512]
            )
            xq[dc][tq] = t
        if tq == 0:
            wv = load_w(wvT, "wv")
            wk = load_w(wkT, "wk")

    # persistent zero-padded V' tiles: [parity][hi], data half written per chunk
    vpads = [[None, None], [None, None]]
    for par in range(2):
        for hi in range(2):
            vt = singles.tile([128, 128], BF, name=f"vp{par}{hi}")
            nc.gpsimd.memset(vt, 0.0)
            vpads[par][hi] = vt

    # zero filler operands: a [128,128] lhsT and [128,512] rhs of zeros
    zpad = singles.tile([128, 128], BF, name="zpad")
    nc.gpsimd.memset(zpad, 0.0)
    zrhs = singles.tile([128, 512], BF, name="zrhs")
    nc.gpsimd.memset(zrhs, 0.0)

    # warm-up: dummy matmuls on the zeroed tiles bridge the DMA latency so the
    # PE's HAM clock gate is released before the first real projection lands.
    wps = sps.tile([128, QB], FP, name="warm", tag="s")
    for i in range(96):
        nc.tensor.matmul(
            wps[:, 0:128], vpads[0][0], vpads[1][0], start=(i == 0), stop=(i == 95)
        )

    # ---- projection emitters (psum borrowed from the S pool tag).
    # Up to two [128,512] blocks share one psum borrow so dribbled projections
    # insert into the S rotation as rarely as possible.
    def eT_block_mms(ws, pair, tt, pshalf):
        for dc in range(N_DC):
            nc.tensor.matmul(
                pshalf,
                ws[dc][:, pair * 128 : (pair + 1) * 128],
                xq[dc][tt],
                start=(dc == 0),
                stop=(dc == N_DC - 1),
            )

    def v_block_mms(tt, pshalf):
        tq, to = divmod(tt, 4)
        for dc in range(N_DC):
            nc.tensor.matmul(
                pshalf,
                xq[dc][tq][:, to * 128 : (to + 1) * 128],
                wv[dc],
                start=(dc == 0),
                stop=(dc == N_DC - 1),
            )

    def project_eT(ws, pair, tts):
        """1-2 t-blocks of a QT/KT pair tile through one psum borrow."""
        ets = QT if ws is wq else KT
        ps = sps.tile([128, QB], FP, name=f"ps_e{pair}_{tts[0]}", tag="s")
        for i, tt in enumerate(tts):
            eT_block_mms(ws, pair, tt, ps[:, i * 512 : (i + 1) * 512])
        et = ets[pair]
        if len(tts) == 2 and tts[1] == tts[0] + 1:
            nc.vector.tensor_copy(et[:, tts[0] * 512 : (tts[0] + 2) * 512], ps)
        else:
            for i, tt in enumerate(tts):
                nc.vector.tensor_copy(
                    et[:, tt * 512 : (tt + 1) * 512], ps[:, i * 512 : (i + 1) * 512]
                )

    def project_v(tts):
        """1-2 V tiles through one psum borrow."""
        ps = sps.tile([128, QB], FP, name=f"ps_v{tts[0]}", tag="s")
        for i, tt in enumerate(tts):
            v_block_mms(tt, ps[:, i * 512 : (i + 1) * 512])
        for i, tt in enumerate(tts):
            v = qkv.tile([128, E], BF, name=f"v{tt}", tag=f"v{tt}")
            nc.vector.tensor_copy(v, ps[:, i * 512 : (i + 1) * 512])
            V[tt] = v

    QT = [None] * N_PAIRS
    KT = [None] * N_PAIRS
    V = [None] * N_KC

    def alloc_pair(p):
        QT[p] = qkv.tile([128, T], BF, name=f"qt{p}", tag=f"qt{p}")
        KT[p] = qkv.tile([128, T], BF, name=f"kt{p}", tag=f"kt{p}")

    # Two-phase design: ALL projections run in the prologue, ordered by the
    # x quarter they consume so the (warm, dense) PE tracks the DMA stream.
    # In-stream projection borrows proved to cost 2-4us of pipeline
    # disruption each, far more than their PE time — so the chunk stream
    # below runs with zero psum-rotation insertions.
    for p in range(N_PAIRS):
        alloc_pair(p)
    for tq in range(N_TQ):
        for p in range(N_PAIRS):
            project_eT(wq, p, (tq,))
            project_eT(wk, p, (tq,))
        project_v((4 * tq, 4 * tq + 1))
        project_v((4 * tq + 2, 4 * tq + 3))

    work = {}

    # ---- attention emitters ----
    def scores_half(p, c, hi):
        """S tiles for one head of the pair: 2x [128k, 1024q] psum."""
        base = hi * 64
        out = {}
        for qb in range(2):
            s = sps.tile([128, QB], FP, name=f"s_{p}_{c}_{hi}_{qb}", tag="s")
            for qt in range(2):
                q0 = qb * QB + qt * 512
                nc.tensor.matmul(
                    s[:, qt * 512 : (qt + 1) * 512],
                    KT[p][base : base + 64, c * 128 : (c + 1) * 128],
                    QT[p][base : base + 64, q0 : q0 + 512],
                    start=True,
                    stop=True,
                    tile_position=(base, 0),
                )
            out[qb] = s
        return out

    def exps_half(p, c, hi, stiles, ptiles, zs):
        """Two exps for one head. qb0's row-sum goes to DVE (tensor_reduce,
        hidden under qb1's exp); qb1 uses the ScalarE accumulator."""
        for qb in range(2):
            pt = pp.tile([128, QB], BF, name=f"p_{p}_{c}_{hi}_{qb}", tag="p")
            nc.scalar.activation(
                out=pt,
                in_=stiles[(hi, qb)],
                func=Exp,
                scale=SCALE,
                accum_out=zs[:, 2 * hi + 1 : 2 * hi + 2] if qb == 1 else None,
            )
            if qb == 0:
                nc.vector.tensor_reduce(
                    zs[:, 2 * hi : 2 * hi + 1], pt, axis=AX, op=ADD
                )
            ptiles[(hi, qb)] = pt

    def zchain_half(p, c, hi, zs):
        """Z -> 1/Z -> scaled V' for one head; needs only that head's exps."""
        za = zp.tile([128, 1], FP, name=f"za_{p}_{c}_{hi}", tag=f"za{hi}")
        nc.vector.tensor_add(za, zs[:, 2 * hi : 2 * hi + 1], zs[:, 2 * hi + 1 : 2 * hi + 2])
        rz = zp.tile([128, 1], FP, name=f"rz_{p}_{c}_{hi}", tag=f"rz{hi}")
        nc.vector.reciprocal(out=rz, in_=za)
        vt = vpads[c % 2][hi]
        lo = hi * 64
        nc.vector.tensor_scalar_mul(
            vt[:, lo : lo + 64],
            V[c][:, p * 128 + lo : p * 128 + lo + 64],
            rz,
        )
        return vt

    def av_half(p, c, acc, vt, ptiles, hi):
        for qb in range(2):
            for qt in range(2):
                nc.tensor.matmul(
                    acc[qb][:, qt * 512 : (qt + 1) * 512],
                    vt,
                    ptiles[(hi, qb)][:, qt * 512 : (qt + 1) * 512],
                    start=(c == 0 and hi == 0),
                    stop=(c == N_KC - 1 and hi == 1),
                )
        if hi == 0:
            # dependency-free zero matmuls (+0.0 into acc): keep the PE busy
            # enough that the HAM clock gate never re-throttles mid-stream.
            for f in range(4):
                nc.tensor.matmul(
                    acc[f % 2][:, 0:512], zpad, zrhs, start=False, stop=False
                )

    # ---- pipelined main loop ----
    stiles = {}
    for hi in range(2):
        for qb, s in scores_half(0, 0, hi).items():
            stiles[(hi, qb)] = s

    for p in range(N_PAIRS):
        acc = [
            accps.tile([128, QB], FP, name=f"acc{qb}_{p}", tag=f"acc{qb}")
            for qb in range(2)
        ]
        for c in range(N_KC):
            zs = zp.tile([128, 4], FP, name=f"zs_{p}_{c}", tag="zs")
            ptiles = {}
            nxt = (p, c + 1) if c + 1 < N_KC else (p + 1, 0)
            # head 0: exps -> Z chain -> next-chunk scores -> AV
            exps_half(p, c, 0, stiles, ptiles, zs)
            vt0 = zchain_half(p, c, 0, zs)
            nstiles = {}
            if nxt[0] < N_PAIRS:
                for qb, s in scores_half(*nxt, 0).items():
                    nstiles[(0, qb)] = s
            av_half(p, c, acc, vt0, ptiles, 0)
            # head 1 likewise, overlapping head 0's AV with its exps
            exps_half(p, c, 1, stiles, ptiles, zs)
            vt1 = zchain_half(p, c, 1, zs)
            if nxt[0] < N_PAIRS:
                for qb, s in scores_half(*nxt, 1).items():
                    nstiles[(1, qb)] = s
            av_half(p, c, acc, vt1, ptiles, 1)
            stiles = nstiles
            for fn in work.get((p, c), []):
                fn()
        # epilogue: outT rows for this pair (host transposes back)
        for qb in range(2):
            ot = op.tile([128, QB], FP, name=f"ot_{p}_{qb}", tag="ot")
            nc.vector.tensor_copy(ot, acc[qb])
            nc.sync.dma_start(
                out=outT[p * 128 : (p + 1) * 128, qb * QB : (qb + 1) * QB], in_=ot
            )

    for pool in (op, zp, pp, accps, sps, qkv, wp, xw, singles):
        pool.release()


def build():
    import concourse.bacc as bacc
    import concourse.mybir as mybir
    import concourse.tile as tile

    nc = bacc.Bacc("TRN2", target_bir_lowering=False, debug=False)
    FP = mybir.dt.float32
    BF = mybir.dt.bfloat16
    xT = nc.dram_tensor("xT", [D, T], BF, kind="ExternalInput").ap()
    wqT = nc.dram_tensor("wqT", [D, E], BF, kind="ExternalInput").ap()
    wkT = nc.dram_tensor("wkT", [D, E], BF, kind="ExternalInput").ap()
    wvT = nc.dram_tensor("wvT", [D, E], BF, kind="ExternalInput").ap()
    outT = nc.dram_tensor("outT", [E, T], FP, kind="ExternalOutput").ap()
    with tile.TileContext(nc) as tc:
        _build_kernel(tc, xT, wqT, wkT, wvT, outT)
    nc.compile()
    _split_multi_waits(nc)
    return nc


def _get_nc():
    global _built
    if _built is None:
        _built = build()
    return _built


def make_in_maps(x, Wq, Wk, Wv):
    import ml_dtypes

    bf16 = ml_dtypes.bfloat16
    in_maps = []
    for c in range(N_CORES):
        b, g = divmod(c, 2)
        e0 = E * g
        in_maps.append(
            {
                "xT": np.ascontiguousarray(x[b].T).astype(bf16),
                "wqT": np.ascontiguousarray(Wq[e0 : e0 + E, :].T).astype(bf16),
                "wkT": np.ascontiguousarray(Wk[e0 : e0 + E, :].T).astype(bf16),
                "wvT": np.ascontiguousarray(Wv[e0 : e0 + E, :].T).astype(bf16),
            }
        )
    return in_maps


def assemble_out(results):
    out = np.empty((B, T, D), np.float32)
    for c in range(N_CORES):
        b, g = divmod(c, 2)
        e0 = E * g
        out[b][:, e0 : e0 + E] = results[c]["outT"].T
    return out


def kernel(x, padding_mask, Wq, Wk, Wv):
    x = np.asarray(x, dtype=np.float32)
    padding_mask = np.asarray(padding_mask, dtype=np.float32)
    Wq = np.asarray(Wq, dtype=np.float32)
    Wk = np.asarray(Wk, dtype=np.float32)
    Wv = np.asarray(Wv, dtype=np.float32)
    if not np.all(padding_mask == 1.0):
        return _np_reference(x, padding_mask, Wq, Wk, Wv)

    from concourse.bass_utils import run_bass_kernel_spmd

    nc = _get_nc()
    in_maps = make_in_maps(x, Wq, Wk, Wv)
    res = run_bass_kernel_spmd(nc, in_maps, list(range(N_CORES)))
    return assemble_out(res.results)


# revision 25
# speedup vs baseline: 1.0632x; 1.0163x over previous
"""Multi-head self-attention (B=4, T=2048, D=1024, H=16) on 8 TRN2 NeuronCores.

Reference quirk: softmax normalizes over the QUERY axis (dim=2 of
[B,H,T1,T2]), i.e. attn[q,k] = exp(s[q,k]) / sum_q' exp(s[q',k]).

Sharding (fully SPMD, one NEFF for all 8 cores):
  core c -> batch b = c//2, head-group g = c%2 (8 heads = 512 cols of Wq/Wk/Wv).
  Host pre-slices AND pre-transposes per-core inputs (xT, wqT/wkT/wvT), runs
  the kernel, and stitches the 8 transposed [512, T] output shards back
  together (host-side transpose: device emits outT, avoiding PE transposes).

Device algorithm per core (v3 — software-pipelined, dense-PE schedule):
  1. x is DMAed as 32 [128,512] quarter-tiles (t-major order) so pair-0
     QT/KT projection can start ~6us in, overlapping the DMA tail.
     Prologue: QT/KT for pairs 0 AND 1 (PE work hidden under the x DMA),
     V[0:4]. Remaining V tiles dribble through pair 0's chunk stream;
     QT/KT of pair p+1 dribble through pair p's stream (p>=1).
  2. Per head-pair, per 128-wide key chunk:
       S = K @ Q^T [128 k, 1024 q] per (head, q-half) in PSUM; the two
       heads' score MMs are interleaved at adjacent tile_position row
       groups (0 / 64) so the PE can stream them concurrently,
       P = exp(SCALE * S) via ScalarE PSUM->SBUF (bf16),
       Z[k] row-sums via DVE tensor_reduce over P (keeps ScalarE lean),
       V'[k,:] = V[k,:] / Z[k] into persistent zero-padded vpad tiles,
       outT[d, q] += vpad^T @ P accumulated over 16 chunks in PSUM.
     Emission is pipelined: scores for chunk c+1 are issued between the
     exp and AV of chunk c so neither PE nor ACT queues behind the other.
  3. Epilogue per pair: acc -> SBUF copy -> DMA to outT rows (no transpose).
"""

import numpy as np

B, T, D, H = 4, 2048, 1024, 16
DH = D // H
SCALE = 1.0 / (DH**0.5)
N_CORES = 8
E = D // 2  # 512 output cols per core (8 heads)
N_PAIRS = 4  # head-pairs per core
N_DC = D // 128  # 8 contraction chunks for projections
N_KC = T // 128  # 16 key chunks
N_TQ = 4  # x quarter-tiles along t
QB = 1024  # exp free-dim block (2 PSUM banks)
V_PRE = 10  # V tiles projected in the prologue; rest dribbled

_built = None  # (nc,) cache so repeat kernel() calls skip rebuild/recompile


def _np_reference(x, padding_mask, Wq, Wk, Wv):
    """Pure-numpy fallback, used only if the mask is not all-ones."""
    x64 = x.astype(np.float64)
    Q = (x64 @ Wq.T.astype(np.float64)).reshape(B, T, H, DH).transpose(0, 2, 1, 3)
    K = (x64 @ Wk.T.astype(np.float64)).reshape(B, T, H, DH).transpose(0, 2, 1, 3)
    V = (x64 @ Wv.T.astype(np.float64)).reshape(B, T, H, DH).transpose(0, 2, 1, 3)
    s = np.einsum("bhqd,bhkd->bhqk", Q, K) * SCALE
    s = np.where(padding_mask[:, None, :, :] == 0, -np.inf, s)
    s = s - s.max(axis=2, keepdims=True)
    p = np.exp(s)
    p = p / p.sum(axis=2, keepdims=True)
    out = np.einsum("bhqk,bhkd->bhqd", p, V)
    return out.transpose(0, 2, 1, 3).reshape(B, T, D).astype(np.float32)


def _split_multi_waits(nc):
    """Walrus caps sync waits at 1 per instruction; Tile's tail drain can carry
    several. Move the extras onto single-wait drains appended to the previous
    basic block (same engine, earlier in program order)."""
    import concourse.mybir as mybir

    blocks = list(nc.m.functions[0].blocks)
    for bi, blk in enumerate(blocks):
        for inst in blk.instructions:
            if type(inst).__name__ not in ("InstDrain", "InstNoOp", "InstEventSemaphore"):
                continue
            si = inst.sync_info
            if si is not None and si.on_wait and len(si.on_wait) > 1:
                waits = list(si.on_wait)
                keep, extra = waits[-1], waits[:-1]
                assert all(w.wait_mode == "sem-ge-imm" for w in extra), extra
                si.on_wait = [keep]
                assert bi > 0, "multi-wait in first block"
                prev = blocks[bi - 1]
                for j, w in enumerate(extra):
                    d = mybir.InstDrain(
                        name=f"{inst.name}-ws{j}",
                        engine=inst.engine,
                        sync_info=mybir.SyncInfo(on_wait=[w], on_update=[]),
                    )
                    prev.add_instruction(d)


def _build_kernel(tc, xT, wqT, wkT, wvT, outT):
    import concourse.bass as bass  # noqa: F401
    import concourse.mybir as mybir

    nc = tc.nc
    FP = mybir.dt.float32
    FR = mybir.dt.float32r
    BF = mybir.dt.bfloat16
    Exp = mybir.ActivationFunctionType.Exp
    AX = mybir.AxisListType.X
    ADD = mybir.AluOpType.add

    # long-lived pools
    singles = tc.alloc_tile_pool(name="singles", bufs=1)
    xw = tc.alloc_tile_pool(name="xw", bufs=1)
    wp = tc.alloc_tile_pool(name="wp", bufs=3)
    qkv = tc.alloc_tile_pool(name="qkv", bufs=1)
    sps = tc.alloc_tile_pool(name="sps", bufs=2, space="PSUM")
    accps = tc.alloc_tile_pool(name="accps", bufs=1, space="PSUM")
    pp = tc.alloc_tile_pool(name="pp", bufs=8)
    zp = tc.alloc_tile_pool(name="zp", bufs=4)
    op = tc.alloc_tile_pool(name="op", bufs=2)

    # ---- loads: wq/wk first (pair-0 projections), x quarters t-major ----
    def load_w(wap, label):
        ws = []
        for dc in range(N_DC):
            t = wp.tile([128, E], BF, name=f"{label}{dc}", tag=f"w{dc}")
            nc.sync.dma_start(out=t, in_=wap[dc * 128 : (dc + 1) * 128, :])
            ws.append(t)
        return ws

    wq = load_w(wqT, "wq")

    xq = [[None] * N_TQ for _ in range(N_DC)]
    wk = wv = None
    for tq in range(N_TQ):
        for dc in range(N_DC):
            t = xw.tile([128, 512], BF, name=f"x{dc}_{tq}", tag=f"x{dc}_{tq}")
            nc.sync.dma_start(
                out=t, in_=xT[dc * 128 : (dc + 1) * 128, tq * 512 : (tq + 1) * 512]
            )
            xq[dc][tq] = t
        if tq == 0:
            wv = load_w(wvT, "wv")
            wk = load_w(wkT, "wk")

    # persistent zero-padded V' tiles: [parity][hi], data half written per chunk
    vpads = [[None, None], [None, None]]
    for par in range(2):
        for hi in range(2):
            vt = singles.tile([128, 128], BF, name=f"vp{par}{hi}")
            nc.gpsimd.memset(vt, 0.0)
            vpads[par][hi] = vt

    # zero filler operands: a [128,128] lhsT and [128,512] rhs of zeros
    zpad = singles.tile([128, 128], BF, name="zpad")
    nc.gpsimd.memset(zpad, 0.0)
    zrhs = singles.tile([128, 512], BF, name="zrhs")
    nc.gpsimd.memset(zrhs, 0.0)

    # warm-up: dummy matmuls on the zeroed tiles bridge the DMA latency so the
    # PE's HAM clock gate is released before the first real projection lands.
    wps = sps.tile([128, QB], FP, name="warm", tag="s")
    for i in range(96):
        nc.tensor.matmul(
            wps[:, 0:128], vpads[0][0], vpads[1][0], start=(i == 0), stop=(i == 95)
        )

    # ---- projection emitters (psum borrowed from the S pool tag).
    # Up to two [128,512] blocks share one psum borrow so dribbled projections
    # insert into the S rotation as rarely as possible.
    def eT_block_mms(ws, pair, tt, pshalf):
        for dc in range(N_DC):
            nc.tensor.matmul(
                pshalf,
                ws[dc][:, pair * 128 : (pair + 1) * 128],
                xq[dc][tt],
                start=(dc == 0),
                stop=(dc == N_DC - 1),
            )

    def v_block_mms(tt, pshalf):
        tq, to = divmod(tt, 4)
        for dc in range(N_DC):
            nc.tensor.matmul(
                pshalf,
                xq[dc][tq][:, to * 128 : (to + 1) * 128],
                wv[dc],
                start=(dc == 0),
                stop=(dc == N_DC - 1),
            )

    def project_eT(ws, pair, tts):
        """1-2 t-blocks of a QT/KT pair tile through one psum borrow."""
        ets = QT if ws is wq else KT
        ps = sps.tile([128, QB], FP, name=f"ps_e{pair}_{tts[0]}", tag="s")
        for i, tt in enumerate(tts):
            eT_block_mms(ws, pair, tt, ps[:, i * 512 : (i + 1) * 512])
        et = ets[pair]
        if len(tts) == 2 and tts[1] == tts[0] + 1:
            nc.vector.tensor_copy(et[:, tts[0] * 512 : (tts[0] + 2) * 512], ps)
        else:
            for i, tt in enumerate(tts):
                nc.vector.tensor_copy(
                    et[:, tt * 512 : (tt + 1) * 512], ps[:, i * 512 : (i + 1) * 512]
                )

    def project_v(tts):
        """1-2 V tiles through one psum borrow."""
        ps = sps.tile([128, QB], FP, name=f"ps_v{tts[0]}", tag="s")
        for i, tt in enumerate(tts):
            v_block_mms(tt, ps[:, i * 512 : (i + 1) * 512])
        for i, tt in enumerate(tts):
            v = qkv.tile([128, E], BF, name=f"v{tt}", tag=f"v{tt}")
            nc.vector.tensor_copy(v, ps[:, i * 512 : (i + 1) * 512])
            V[tt] = v

    QT = [None] * N_PAIRS
    KT = [None] * N_PAIRS
    V = [None] * N_KC

    def alloc_pair(p):
        QT[p] = qkv.tile([128, T], BF, name=f"qt{p}", tag=f"qt{p}")
        KT[p] = qkv.tile([128, T], BF, name=f"kt{p}", tag=f"kt{p}")

    # prologue: pair-0 QT (all queries) + KT-t0 + V[0:V_PRE], gated only on
    # what each block's x quarter needs so the PE tracks the DMA stream.
    alloc_pair(0)
    project_eT(wq, 0, (0,))
    project_eT(wk, 0, (0,))
    project_v((0, 1))
    project_v((2, 3))
    project_v((4, 5))
    project_v((6, 7))
    project_eT(wq, 0, (1,))
    project_v((8, 9))
    project_v((10, 11))
    project_eT(wq, 0, (2,))
    project_eT(wq, 0, (3,))

    # dribble schedule: single-block borrows, at most one per chunk, kept off
    # the fragile pair-transition chunks. Own KT t1-3 land just before first
    # use; the next pair's QT + KT-t0 fill c=9..13.
    work = {}

    def add_work(p, c, fn):
        work.setdefault((p, c), []).append(fn)

    def proj_item(ws, q, tts):
        return lambda: project_eT(ws, q, tts)

    for p in range(N_PAIRS):
        add_work(p, 2, proj_item(wk, p, (1,)))
        add_work(p, 5, proj_item(wk, p, (2,)))
        add_work(p, 8, proj_item(wk, p, (3,)))
        if p < N_PAIRS - 1:
            add_work(p, 8, (lambda q: (lambda: alloc_pair(q)))(p + 1))
            add_work(p, 9, proj_item(wq, p + 1, (0,)))
            add_work(p, 10, proj_item(wq, p + 1, (1,)))
            add_work(p, 11, proj_item(wq, p + 1, (2,)))
            add_work(p, 12, proj_item(wq, p + 1, (3,)))
            add_work(p, 13, proj_item(wk, p + 1, (0,)))
    for i, c in enumerate((3, 4, 6, 7)):
        tt = 12 + i  # V[12..15] during pair 0, just-in-time
        add_work(0, c, (lambda t2: (lambda: project_v((t2,))))(tt))

    # ---- attention emitters ----
    def scores_half(p, c, hi):
        """S tiles for one head of the pair: 2x [128k, 1024q] psum."""
        base = hi * 64
        out = {}
        for qb in range(2):
            s = sps.tile([128, QB], FP, name=f"s_{p}_{c}_{hi}_{qb}", tag="s")
            for qt in range(2):
                q0 = qb * QB + qt * 512
                nc.tensor.matmul(
                    s[:, qt * 512 : (qt + 1) * 512],
                    KT[p][base : base + 64, c * 128 : (c + 1) * 128],
                    QT[p][base : base + 64, q0 : q0 + 512],
                    start=True,
                    stop=True,
                    tile_position=(base, 0),
                )
            out[qb] = s
        return out

    def exps_half(p, c, hi, stiles, ptiles, zs):
        """Two exps for one head. qb0's row-sum goes to DVE (tensor_reduce,
        hidden under qb1's exp); qb1 uses the ScalarE accumulator."""
        for qb in range(2):
            pt = pp.tile([128, QB], BF, name=f"p_{p}_{c}_{hi}_{qb}", tag="p")
            nc.scalar.activation(
                out=pt,
                in_=stiles[(hi, qb)],
                func=Exp,
                scale=SCALE,
                accum_out=zs[:, 2 * hi + 1 : 2 * hi + 2] if qb == 1 else None,
            )
            if qb == 0:
                nc.vector.tensor_reduce(
                    zs[:, 2 * hi : 2 * hi + 1], pt, axis=AX, op=ADD
                )
            ptiles[(hi, qb)] = pt

    def zchain_half(p, c, hi, zs):
        """Z -> 1/Z -> scaled V' for one head; needs only that head's exps."""
        za = zp.tile([128, 1], FP, name=f"za_{p}_{c}_{hi}", tag=f"za{hi}")
        nc.vector.tensor_add(za, zs[:, 2 * hi : 2 * hi + 1], zs[:, 2 * hi + 1 : 2 * hi + 2])
        rz = zp.tile([128, 1], FP, name=f"rz_{p}_{c}_{hi}", tag=f"rz{hi}")
        nc.vector.reciprocal(out=rz, in_=za)
        vt = vpads[c % 2][hi]
        lo = hi * 64
        nc.vector.tensor_scalar_mul(
            vt[:, lo : lo + 64],
            V[c][:, p * 128 + lo : p * 128 + lo + 64],
            rz,
        )
        return vt

    def av_half(p, c, acc, vt, ptiles, hi):
        for qb in range(2):
            for qt in range(2):
                nc.tensor.matmul(
                    acc[qb][:, qt * 512 : (qt + 1) * 512],
                    vt,
                    ptiles[(hi, qb)][:, qt * 512 : (qt + 1) * 512],
                    start=(c == 0 and hi == 0),
                    stop=(c == N_KC - 1 and hi == 1),
                )
        if hi == 0:
            # dependency-free zero matmuls (+0.0 into acc): keep the PE busy
            # enough that the HAM clock gate never re-throttles mid-stream.
            for f in range(4):
                nc.tensor.matmul(
                    acc[f % 2][:, 0:512], zpad, zrhs, start=False, stop=False
                )

    # ---- pipelined main loop ----
    stiles = {}
    for hi in range(2):
        for qb, s in scores_half(0, 0, hi).items():
            stiles[(hi, qb)] = s

    for p in range(N_PAIRS):
        acc = [
            accps.tile([128, QB], FP, name=f"acc{qb}_{p}", tag=f"acc{qb}")
            for qb in range(2)
        ]
        for c in range(N_KC):
            zs = zp.tile([128, 4], FP, name=f"zs_{p}_{c}", tag="zs")
            ptiles = {}
            nxt = (p, c + 1) if c + 1 < N_KC else (p + 1, 0)
            # head 0: exps -> Z chain -> next-chunk scores -> AV
            exps_half(p, c, 0, stiles, ptiles, zs)
            vt0 = zchain_half(p, c, 0, zs)
            nstiles = {}
            if nxt[0] < N_PAIRS:
                for qb, s in scores_half(*nxt, 0).items():
                    nstiles[(0, qb)] = s
            av_half(p, c, acc, vt0, ptiles, 0)
            # head 1 likewise, overlapping head 0's AV with its exps
            exps_half(p, c, 1, stiles, ptiles, zs)
            vt1 = zchain_half(p, c, 1, zs)
            if nxt[0] < N_PAIRS:
                for qb, s in scores_half(*nxt, 1).items():
                    nstiles[(1, qb)] = s
            av_half(p, c, acc, vt1, ptiles, 1)
            stiles = nstiles
            for fn in work.get((p, c), []):
                fn()
        # epilogue: outT rows for this pair (host transposes back)
        for qb in range(2):
            ot = op.tile([128, QB], FP, name=f"ot_{p}_{qb}", tag="ot")
            nc.vector.tensor_copy(ot, acc[qb])
            nc.sync.dma_start(
                out=outT[p * 128 : (p + 1) * 128, qb * QB : (qb + 1) * QB], in_=ot
            )

    for pool in (op, zp, pp, accps, sps, qkv, wp, xw, singles):
        pool.release()


def build():
    import concourse.bacc as bacc
    import concourse.mybir as mybir
    import concourse.tile as tile

    nc = bacc.Bacc("TRN2", target_bir_lowering=False, debug=False)
    FP = mybir.dt.float32
    BF = mybir.dt.bfloat16
    xT = nc.dram_tensor("xT", [D, T], BF, kind="ExternalInput").ap()
    wqT = nc.dram_tensor("wqT", [D, E], BF, kind="ExternalInput").ap()
    wkT = nc.dram_tensor("wkT", [D, E], BF, kind="ExternalInput").ap()
    wvT = nc.dram_tensor("wvT", [D, E], BF, kind="ExternalInput").ap()
    outT = nc.dram_tensor("outT", [E, T], FP, kind="ExternalOutput").ap()
    with tile.TileContext(nc) as tc:
        _build_kernel(tc, xT, wqT, wkT, wvT, outT)
    nc.compile()
    _split_multi_waits(nc)
    return nc


def _get_nc():
    global _built
    if _built is None:
        _built = build()
    return _built


def make_in_maps(x, Wq, Wk, Wv):
    import ml_dtypes

    bf16 = ml_dtypes.bfloat16
    in_maps = []
    for c in range(N_CORES):
        b, g = divmod(c, 2)
        e0 = E * g
        in_maps.append(
            {
                "xT": np.ascontiguousarray(x[b].T).astype(bf16),
                "wqT": np.ascontiguousarray(Wq[e0 : e0 + E, :].T).astype(bf16),
                "wkT": np.ascontiguousarray(Wk[e0 : e0 + E, :].T).astype(bf16),
                "wvT": np.ascontiguousarray(Wv[e0 : e0 + E, :].T).astype(bf16),
            }
        )
    return in_maps


def assemble_out(results):
    out = np.empty((B, T, D), np.float32)
    for c in range(N_CORES):
        b, g = divmod(c, 2)
        e0 = E * g
        out[b][:, e0 : e0 + E] = results[c]["outT"].T
    return out


def kernel(x, padding_mask, Wq, Wk, Wv):
    x = np.asarray(x, dtype=np.float32)
    padding_mask = np.asarray(padding_mask, dtype=np.float32)
    Wq = np.asarray(Wq, dtype=np.float32)
    Wk = np.asarray(Wk, dtype=np.float32)
    Wv = np.asarray(Wv, dtype=np.float32)
    if not np.all(padding_mask == 1.0):
        return _np_reference(x, padding_mask, Wq, Wk, Wv)

    from concourse.bass_utils import run_bass_kernel_spmd

    nc = _get_nc()
    in_maps = make_in_maps(x, Wq, Wk, Wv)
    res = run_bass_kernel_spmd(nc, in_maps, list(range(N_CORES)))
    return assemble_out(res.results)
